# revision 4
# baseline (speedup 1.0000x reference)
"""Trainium2 Bass kernel for MockBitNetLayer:

    scale = mean(|W|, axis=1)            # [O, 1]
    y = x @ (sign(W) * scale).T + bias   # [T, O]

Strategy (column-parallel over 8 NeuronCores):
  - Each core owns an O/8 = 2048-row shard of W and bias, and reads the
    full x.  Outputs are the feature-axis shards of y, kept transposed
    on-device (yt = y.T shard, [O_shard, T]) so the per-row scale/bias
    are per-partition and can be fused into the PSUM eviction on the
    scalar engine.  The host concatenates the shards and transposes.
  - sign(W) is exactly representable in bf16, so the matmul runs in
    bf16 (1 PE cycle/row vs 4 for fp32).  The only approximation is
    rounding x to bf16 (~2e-3 rel err).  An optional "split" mode also
    accumulates the bf16 residual of x (x_lo = x - bf16(x)) for
    ~1e-6 rel err at 2x the PE time.
  - K (d_in) must be on partitions for both matmul operands, so both
    sign(W) and x tiles are transposed on-chip with PE transposes
    (fp32, 2 cycles/row) and evicted to SBUF as bf16.
  - sign(W).T stays fully resident in SBUF ([128, 32, 2048] bf16 =
    128 KiB/partition); x.T is staged one 512-token slab at a time.
"""

import os
import sys

for _p in ("/opt/trn_rl_repo", "/root/.axon_site/_ro/trn_rl_repo"):
    if os.path.isdir(_p) and _p not in sys.path:
        sys.path.insert(0, _p)

import numpy as np

import concourse.bacc as bacc
import concourse.mybir as mybir
import concourse.tile as tile
from concourse.bass import ds
from concourse.bass_utils import run_bass_kernel_spmd
from concourse.masks import make_identity

P = 128
N_CORES = 8

# Full problem shapes (hardcoded per spec).
T_FULL = 8192
K_FULL = 4096
O_FULL = 16384


def build_kernel_body(tc, x, w, b, yt, T, K, O, TCH=512, split=False):
    """Emit the per-core program.

    x:  [T, K] f32 (replicated)        w: [O, K] f32 (shard)
    b:  [O]    f32 (shard)             yt: [O, T] f32 out (shard of y.T)
    """
    nc = tc.nc
    f32 = mybir.dt.float32
    bf16 = mybir.dt.bfloat16

    KT = K // P        # k tiles (partition-dim tiles of the contraction)
    OT = O // P        # o tiles
    NTCH = T // TCH    # token chunks
    KS = K // 512      # 512-wide k slabs per row tile
    OB = 4 if OT % 4 == 0 else (2 if OT % 2 == 0 else 1)  # o tiles per psum block
    NOB = OT // OB
    TSUB = TCH // P    # 128-row subtiles per token chunk

    with (
        tc.tile_pool(name="const", bufs=1) as const_pool,
        tc.tile_pool(name="stage", bufs=4) as stage,
        tc.tile_pool(name="swt", bufs=1) as swt_pool,
        tc.tile_pool(name="xt", bufs=1) as xt_pool,
        tc.tile_pool(name="out", bufs=4) as out_pool,
        tc.tile_pool(name="psum_t", bufs=3, space="PSUM") as psum_t,
        tc.tile_pool(name="psum_mm", bufs=5, space="PSUM") as psum_mm,
    ):
        ident = const_pool.tile([P, P], f32)
        make_identity(nc, ident)

        scale_sb = const_pool.tile([P, OT], f32)
        bias_sb = const_pool.tile([P, OT], f32)
        partials = const_pool.tile([P, KS], f32)

        # ---- W prep: scale, sign, transpose; sign(W).T resident in SBUF ----
        swt = swt_pool.tile([P, KT, O], bf16)
        for ot in range(OT):
            nc.sync.dma_start(
                bias_sb[:, ot : ot + 1],
                b[ds(ot * P, P)].rearrange("(p one) -> p one", one=1),
            )
            for ks in range(KS):
                ws = stage.tile([P, 512], f32, tag="stage_f32")
                nc.sync.dma_start(ws, w[ds(ot * P, P), ds(ks * 512, 512)])
                # per-row |W| partial sums
                nc.vector.tensor_reduce(
                    out=partials[:, ks : ks + 1],
                    in_=ws,
                    axis=mybir.AxisListType.X,
                    op=mybir.AluOpType.add,
                    apply_absolute_value=True,
                )
                # transpose the four 128x128 blocks of this slab
                pt = psum_t.tile([P, 4, P], f32, tag="pt")
                for j in range(4):
                    nc.tensor.transpose(pt[:, j, :], ws[:, ds(j * P, P)], ident)
                # sign() on eviction, cast to bf16, into resident swt
                nc.scalar.sign(
                    swt[:, ds(ks * 4, 4), ds(ot * P, P)],
                    pt,
                )
            # scale[o] = sum(partials) / K
            stot = const_pool.tile([P, 1], f32, tag="stot")
            nc.vector.tensor_reduce(
                out=stot,
                in_=partials,
                axis=mybir.AxisListType.X,
                op=mybir.AluOpType.add,
            )
            nc.scalar.mul(scale_sb[:, ot : ot + 1], stot, 1.0 / K)

        # ---- main loop over token chunks ----
        n_pass = 2 if split else 1
        for tc_i in range(NTCH):
            # build x.T slab(s) for this chunk: [P, KT, TCH] bf16
            xt = xt_pool.tile([P, KT, TCH], bf16, tag="xt_hi")
            if split:
                xt_lo = xt_pool.tile([P, KT, TCH], bf16, tag="xt_lo")
            for ts_i in range(TSUB):
                for ks in range(KS):
                    xs = stage.tile([P, 512], f32, tag="stage_f32")
                    nc.sync.dma_start(
                        xs,
                        x[ds(tc_i * TCH + ts_i * P, P), ds(ks * 512, 512)],
                    )
                    pt = psum_t.tile([P, 4, P], f32, tag="pt")
                    for j in range(4):
                        nc.tensor.transpose(pt[:, j, :], xs[:, ds(j * P, P)], ident)
                    dst = xt[:, ds(ks * 4, 4), ds(ts_i * P, P)]
                    nc.vector.tensor_copy(dst, pt)
                    if split:
                        # residual: x - bf16(x), rounded to bf16
                        ptf = stage.tile([P, 4, P], f32, tag="stage_res")
                        nc.vector.tensor_sub(ptf, pt, dst)
                        nc.vector.tensor_copy(
                            xt_lo[:, ds(ks * 4, 4), ds(ts_i * P, P)], ptf
                        )
            # matmuls: psum[o_tile 128, TCH] accumulated over KT (x n_pass)
            for ob in range(NOB):
                psums = [
                    psum_mm.tile([P, TCH], f32, tag="acc", name=f"acc{oi}")
                    for oi in range(OB)
                ]
                for k in range(KT):
                    for pi in range(n_pass):
                        rhs_t = xt if pi == 0 else xt_lo
                        for oi in range(OB):
                            ot = ob * OB + oi
                            nc.tensor.matmul(
                                psums[oi],
                                lhsT=swt[:, k, ds(ot * P, P)],
                                rhs=rhs_t[:, k, :],
                                start=(k == 0 and pi == 0),
                                stop=(k == KT - 1 and pi == n_pass - 1),
                            )
                for oi in range(OB):
                    ot = ob * OB + oi
                    out_sb = out_pool.tile([P, TCH], f32)
                    # y.T[o, t] = psum * scale[o] + bias[o]
                    nc.scalar.activation(
                        out_sb,
                        psums[oi],
                        mybir.ActivationFunctionType.Identity,
                        bias=bias_sb[:, ot : ot + 1],
                        scale=scale_sb[:, ot : ot + 1],
                    )
                    nc.sync.dma_start(
                        yt[ds(ot * P, P), ds(tc_i * TCH, TCH)], out_sb
                    )


def build_bass(T=T_FULL, K=K_FULL, O=O_FULL // N_CORES, TCH=512, split=False):
    nc = bacc.Bacc(trn_type="TRN2")
    f32 = mybir.dt.float32
    x = nc.dram_tensor("x", [T, K], f32, kind="ExternalInput").ap()
    w = nc.dram_tensor("w", [O, K], f32, kind="ExternalInput").ap()
    b = nc.dram_tensor("b", [O], f32, kind="ExternalInput").ap()
    yt = nc.dram_tensor("yt", [O, T], f32, kind="ExternalOutput").ap()
    with tile.TileContext(nc) as tc:
        build_kernel_body(tc, x, w, b, yt, T, K, O, TCH=TCH, split=split)
    nc.finalize()
    return nc


_CACHED_NC = None


def _get_nc():
    global _CACHED_NC
    if _CACHED_NC is None:
        _CACHED_NC = build_bass()
    return _CACHED_NC


def make_in_maps(x, weight, bias):
    x = np.ascontiguousarray(np.asarray(x, dtype=np.float32))
    weight = np.ascontiguousarray(np.asarray(weight, dtype=np.float32))
    bias = np.ascontiguousarray(np.asarray(bias, dtype=np.float32))
    O = weight.shape[0] // N_CORES
    return [
        {
            "x": x,
            "w": weight[c * O : (c + 1) * O],
            "b": bias[c * O : (c + 1) * O],
        }
        for c in range(N_CORES)
    ]


def kernel(x, weight, bias):
    nc = _get_nc()
    in_maps = make_in_maps(x, weight, bias)
    res = run_bass_kernel_spmd(nc, in_maps, list(range(N_CORES)))
    yt = np.concatenate([r["yt"] for r in res.results], axis=0)  # [O_FULL, T]
    return np.ascontiguousarray(yt.T)
